# revision 5
# baseline (speedup 1.0000x reference)
"""Trainium2 Bass kernel for nn_Blender_70334384439403 (contrastive loss_fn).

Reference computation (per group g in {real, fake}):
    f = feats[n] viewed as [C=128, HW=784], unit-normalized over C per pixel
    pos = pos_thr * f ; neg = neg_thr * f          (per-pixel binary masks)
    sim[n] = pos^T @ neg / T                        ([HW, HW] per instance)
    l[n] = sum(exp(sim[n]))
    out = -log(s_real / (s_real + s_fake)),  s_* = sum_n l_*[n]

Sharding: data-parallel over instances; each of the 8 cores gets 8 real +
8 fake instances and returns per-partition partial exp-sums [128, 16].
Host epilogue sums partials and applies the final -log ratio (scalar).

Per-core kernel strategy (dense):
    sim/T = raw[h,w] * s_pos[h] * s_neg[w]
      raw  = f^T f in bf16 on PE, native [C, HW] layout (no transposes)
      s    = mask * 1/(norm * sqrt(T)); computed for all 8 instances of a
             group at once: norm^2 rows batched into one PSUM tile via
             selection-matrix matmuls, then exp(-0.5*ln(x) - 0.5*ln(T)) on
             ACT (Ln and Exp share one activation table set)
      s_neg folded into the moving matmul operand (gpsimd partition
             broadcast + DVE multiply)
      s_pos applied as the per-partition `scale` of the fused
             exp+accumulate ACT instruction (s_pos transposed to column
             layout via 7 small PE transposes per group)
"""

import math
import os
import sys

import numpy as np

for _p in ("/opt/trn_rl_repo", "/root/.axon_site/_ro/trn_rl_repo"):
    if os.path.isdir(_p) and _p not in sys.path:
        sys.path.insert(0, _p)

from contextlib import ExitStack

import concourse.bass as bass
import concourse.tile as tile
from concourse import bacc, mybir
from concourse import masks as bass_masks
from concourse.bass_utils import run_bass_kernel_spmd

N_CORES = 8
NPC = 8          # instances per core per group (64 / 8)
C = 128          # channels (contraction dim)
HW = 784         # 28*28 pixels
T = 0.7          # temperature (same for real and fake)
M_TILES = [128, 128, 128, 128, 128, 128, 16]   # 784 = 6*128 + 16
N_SPLIT = [(0, 512), (512, 784)]               # psum bank boundary at 512

F32 = mybir.dt.float32
BF16 = mybir.dt.bfloat16

_COMPILED = None
LAST_RESULTS = None


def _build_kernel():
    nc = bacc.Bacc(
        "TRN2",
        target_bir_lowering=False,
        debug=False,
        enable_asserts=False,
        num_devices=N_CORES,
    )
    feats_ap = [
        nc.dram_tensor("rf", [NPC, C, HW], F32, kind="ExternalInput").ap(),
        nc.dram_tensor("ff", [NPC, C, HW], F32, kind="ExternalInput").ap(),
    ]
    masks_ap = [
        (
            nc.dram_tensor("rp", [NPC, HW], F32, kind="ExternalInput").ap(),
            nc.dram_tensor("rn", [NPC, HW], F32, kind="ExternalInput").ap(),
        ),
        (
            nc.dram_tensor("fp", [NPC, HW], F32, kind="ExternalInput").ap(),
            nc.dram_tensor("fn", [NPC, HW], F32, kind="ExternalInput").ap(),
        ),
    ]
    out_ap = nc.dram_tensor("out", [128, 2 * NPC], F32, kind="ExternalOutput").ap()

    with tile.TileContext(nc, trace_sim=False) as tc:
        _emit(tc, out_ap, feats_ap, masks_ap)

    nc.compile()
    return nc


def _emit(tc, out_ap, feats_ap, masks_ap):
    nc = tc.nc
    with ExitStack() as ctx:
        ep = ctx.enter_context

        const_pool = ep(tc.tile_pool(name="const", bufs=1))
        identity = const_pool.tile([128, 128], BF16)
        bass_masks.make_identity(nc, identity[:])
        # sel8[p, i, j] = 1.0 iff i == j; slice [:, i, :] is the [K=128, M=8]
        # stationary that routes instance i's column sums to psum partition i.
        sel8 = const_pool.tile([128, NPC, NPC], BF16)
        nc.gpsimd.memset(sel8[:], 0.0)
        nc.gpsimd.affine_select(
            out=sel8[:],
            in_=sel8[:],
            compare_op=mybir.AluOpType.not_equal,
            fill=1.0,
            base=0,
            pattern=[[-1, NPC], [1, NPC]],
            channel_multiplier=0,
        )
        acc_all = const_pool.tile([128, 2 * NPC], F32)
        # bias tile for the inv-norm Exp: -0.5*ln(T)
        inv_bias = const_pool.tile([128, 1], F32)
        nc.gpsimd.memset(inv_bias[:], -0.5 * math.log(T))

        f32_pool = ep(tc.tile_pool(name="f32", bufs=3))
        fbf_pool = ep(tc.tile_pool(name="fbf", bufs=2 * NPC))
        f2_pool = ep(tc.tile_pool(name="f2", bufs=2))
        mask_pool = ep(tc.tile_pool(name="mask", bufs=4))
        small_pool = ep(tc.tile_pool(name="small", bufs=4))
        spt_pool = ep(tc.tile_pool(name="spt", bufs=2))
        bcast_pool = ep(tc.tile_pool(name="bcast", bufs=2))
        rhss_pool = ep(tc.tile_pool(name="rhss", bufs=2))
        expo_pool = ep(tc.tile_pool(name="expo", bufs=2))
        accin_pool = ep(tc.tile_pool(name="accin", bufs=2))

        pmm_pool = ep(tc.tile_pool(name="pmm", bufs=2, space="PSUM"))
        pnorm_pool = ep(tc.tile_pool(name="pnorm", bufs=1, space="PSUM"))
        ptr_pool = ep(tc.tile_pool(name="ptr", bufs=2, space="PSUM"))

        ln_t = math.log(T)

        for g in range(2):
            pos_m = mask_pool.tile([NPC, HW], F32, tag="mask")
            neg_m = mask_pool.tile([NPC, HW], F32, tag="mask")
            nc.sync.dma_start(pos_m[:], masks_ap[g][0][:])
            nc.sync.dma_start(neg_m[:], masks_ap[g][1][:])

            # ---- pass 1: load features, norms for all NPC instances ----
            fbf = []
            pnorm = pnorm_pool.tile([NPC, HW], F32)
            for i in range(NPC):
                f32t = f32_pool.tile([C, HW], F32, tag="f32")
                nc.sync.dma_start(f32t[:], feats_ap[g][i])
                fb = fbf_pool.tile([C, HW], BF16, tag="fbf")
                nc.vector.tensor_copy(fb[:], f32t[:])
                fbf.append(fb)
                f2 = f2_pool.tile([C, HW], BF16, tag="f2")
                nc.vector.tensor_mul(f2[:], fb[:], fb[:])
                for (n0, n1) in N_SPLIT:
                    nc.tensor.matmul(
                        pnorm[:, n0:n1],
                        lhsT=sel8[:, i, :],
                        rhs=f2[:, n0:n1],
                        start=(i == 0),
                        stop=(i == NPC - 1),
                    )

            # inv = 1/(norm * sqrt(T)) = exp(-0.5*ln(norm^2) - 0.5*ln(T))
            lnn = small_pool.tile([NPC, HW], F32, tag="small")
            nc.scalar.activation(lnn[:], pnorm[:], mybir.ActivationFunctionType.Ln)
            inv = small_pool.tile([NPC, HW], F32, tag="small")
            nc.scalar.activation(
                inv[:],
                lnn[:],
                mybir.ActivationFunctionType.Exp,
                scale=-0.5,
                bias=inv_bias[0:NPC, 0:1],
            )
            s_pos = small_pool.tile([NPC, HW], BF16, tag="ssmall")
            nc.vector.tensor_mul(s_pos[:], inv[:], pos_m[:])
            s_neg = small_pool.tile([NPC, HW], BF16, tag="ssmall")
            nc.vector.tensor_mul(s_neg[:], inv[:], neg_m[:])

            # transpose s_pos [NPC, HW] -> spt [128, 7, NPC] (column layout)
            spt = spt_pool.tile([128, len(M_TILES), NPC], F32)
            moff = 0
            for t, mt in enumerate(M_TILES):
                ptr = ptr_pool.tile([128, NPC], BF16, tag="ptr")
                nc.tensor.transpose(
                    ptr[0:mt, :], s_pos[:, moff : moff + mt], identity[0:NPC, 0:NPC]
                )
                nc.vector.tensor_copy(spt[0:mt, t, :], ptr[0:mt, :])
                moff += mt

            # ---- pass 2: per-instance gram matrix + fused exp-accumulate ----
            for i in range(NPC):
                # partition_broadcast needs its source at partition 0
                sn_row = bcast_pool.tile([1, HW], BF16, tag="snrow")
                nc.sync.dma_start(sn_row[:], s_neg[i : i + 1, :])
                sb = bcast_pool.tile([C, HW], BF16, tag="bcast")
                nc.gpsimd.partition_broadcast(sb[:], sn_row[:])
                rhs_s = rhss_pool.tile([C, HW], BF16, tag="rhss")
                nc.vector.tensor_mul(rhs_s[:], fbf[i][:], sb[:])

                accin = accin_pool.tile([128, len(M_TILES)], F32, tag="accin")
                nc.gpsimd.memset(accin[:], 0.0)
                moff = 0
                for t, mt in enumerate(M_TILES):
                    pmm = pmm_pool.tile([128, 1024], F32, tag="pmm")
                    for (n0, n1) in N_SPLIT:
                        nc.tensor.matmul(
                            pmm[0:mt, n0:n1],
                            lhsT=fbf[i][:, moff : moff + mt],
                            rhs=rhs_s[:, n0:n1],
                            start=True,
                            stop=True,
                        )
                    eo = expo_pool.tile([128, HW], BF16, tag="expo")
                    nc.scalar.activation(
                        eo[0:mt, :],
                        pmm[0:mt, 0:HW],
                        mybir.ActivationFunctionType.Exp,
                        scale=spt[0:mt, t, i : i + 1],
                        accum_out=accin[0:mt, t : t + 1],
                    )
                    moff += mt

                nc.vector.tensor_reduce(
                    acc_all[:, g * NPC + i : g * NPC + i + 1],
                    accin[:],
                    axis=mybir.AxisListType.X,
                    op=mybir.AluOpType.add,
                )

        nc.sync.dma_start(out_ap[:], acc_all[:])


def _get_compiled():
    global _COMPILED
    if _COMPILED is None:
        _COMPILED = _build_kernel()
    return _COMPILED


def kernel(real_feats, fake_feats, real_pos_thr, real_neg_thr,
           fake_pos_thr, fake_neg_thr):
    global LAST_RESULTS
    nc = _get_compiled()

    rf = np.asarray(real_feats, np.float32).reshape(N_CORES * NPC, C, HW)
    ff = np.asarray(fake_feats, np.float32).reshape(N_CORES * NPC, C, HW)
    rp = np.asarray(real_pos_thr, np.float32).reshape(N_CORES * NPC, HW)
    rn = np.asarray(real_neg_thr, np.float32).reshape(N_CORES * NPC, HW)
    fp = np.asarray(fake_pos_thr, np.float32).reshape(N_CORES * NPC, HW)
    fn = np.asarray(fake_neg_thr, np.float32).reshape(N_CORES * NPC, HW)

    in_maps = []
    for cid in range(N_CORES):
        sl = slice(NPC * cid, NPC * (cid + 1))
        in_maps.append({
            "rf": np.ascontiguousarray(rf[sl]),
            "ff": np.ascontiguousarray(ff[sl]),
            "rp": np.ascontiguousarray(rp[sl]),
            "rn": np.ascontiguousarray(rn[sl]),
            "fp": np.ascontiguousarray(fp[sl]),
            "fn": np.ascontiguousarray(fn[sl]),
        })

    res = run_bass_kernel_spmd(nc, in_maps, list(range(N_CORES)))
    LAST_RESULTS = res

    s_real = 0.0
    s_fake = 0.0
    for r in res.results:
        o = r["out"].astype(np.float64)
        s_real += o[:, 0:NPC].sum()
        s_fake += o[:, NPC : 2 * NPC].sum()
    val = -np.log(s_real / (s_fake + s_real))
    return np.array(val, dtype=np.float32)
